# revision 25
# baseline (speedup 1.0000x reference)
"""Multi-head attention (B=2, S=2048, nx=768, H=12) on 8 TRN2 NeuronCores.

Sharding: 24 (batch, head) pairs -> 3 heads per core. Core c handles batch
c//4, heads {3*(c%4), +1, +2}. Each core computes QKV projection for its
head slice, attention, and a partial output projection (its 192 rows of
w_proj); the host sums the 4 partials per batch and adds the bias.

Schedule (~189us vs 241us baseline): the scalar-engine exp stream (96 x
[128,1024] activations, ~20% offloaded to the DVE) is the metronome; all
PE work hides inside it. Key points:
  - contraction is 6x128 (= nx) with no bias row: v-bias folds exactly into
    the host-side output bias (b_eff = b_proj + b_v @ w_proj); q/k biases
    are added during the PSUM->SBUF copies as per-partition scalars.
  - all stationary operands are padded to full 128 partitions / 128
    columns so every matmul keeps Fast Weight Load (K=64 operands pay
    ~110ns of exposed LDWEIGHTS otherwise): kA=[K_even;0], kB=[0;K_odd]
    with q duplicated into both halves, and the PV lhsT view spans
    v|ones|next-head-spill (PSUM rows 65:128 take junk, never read).
  - a fifth of the exps run on the DVE as a one-instruction Schraudolph
    approximation (f32->int16 round-to-nearest affine = bf16 exp bits,
    ~2% rms), which the accuracy budget absorbs (rel err 8.8e-3 vs 2e-2).
  - chunked input DMAs across both HWDGE queues (sync + scalar); v_proj /
    qk_proj(1,2) fill the head-0 scores window; head-2 PV (queries 0:1024)
    lags four chunks behind its exps inside the head-1 window; the
    remaining PV runs qc-merged with the output projection interleaved.
  - norms batch 1/sumexp rows at 32-aligned partitions into one fast
    reciprocal; the PV PSUM is freed by two copies (DVE/ACT alternating)
    before the broadcast multiply, so the next window's accumulation can
    claim the banks immediately.
  - output is staged bf16 (halves the write traffic); the host sums the
    four per-batch partials in f32.
"""

import numpy as np
import ml_dtypes

import concourse.bass as bass
import concourse.tile as tile
import concourse.mybir as mybir
from concourse import bacc

BF16 = mybir.dt.bfloat16
F32 = mybir.dt.float32

NX = 768
D = 64
HPC = 3          # heads per core
N_CORES = 8
KCH = 6          # contraction chunks of 128 (= nx, no bias row)
KDIM = KCH * 128  # 768


def build_nc(S=2048):
    """Build the single-core SPMD program. S = sequence length."""
    TC = S // 128    # t (key) chunks
    QC = S // 512    # q chunks of 512
    nc = bacc.Bacc("TRN2", target_bir_lowering=False, debug=False)

    xt_d = nc.dram_tensor("xt", [KDIM, S], BF16, kind="ExternalInput")
    wqk_d = nc.dram_tensor("wqk", [KDIM, 6 * D], BF16, kind="ExternalInput")
    wv_d = nc.dram_tensor("wv", [KDIM, HPC * D], BF16, kind="ExternalInput")
    wp_d = nc.dram_tensor("wp", [HPC * D, NX], BF16, kind="ExternalInput")
    bq_d = nc.dram_tensor("bq", [128, HPC], F32, kind="ExternalInput")
    bk_d = nc.dram_tensor("bk", [128, HPC], F32, kind="ExternalInput")
    out_d = nc.dram_tensor("out", [S, NX], BF16, kind="ExternalOutput")

    with tile.TileContext(nc) as tc:
        _build_body(tc, out_d.ap(), xt_d.ap(), wqk_d.ap(), wv_d.ap(),
                    wp_d.ap(), bq_d.ap(), bk_d.ap(), S, TC, QC)
    nc.compile()
    return nc


def _build_body(tc, out_d, xt_d, wqk_d, wv_d, wp_d, bq_d, bk_d, S, TC, QC):
    nc = tc.nc
    P = 128
    NHALF = S // 1024  # exp calls per t-chunk, each [128, 1024]

    with tc.tile_pool(name="const", bufs=1) as cpool, \
         tc.tile_pool(name="epoolA", bufs=TC + 2) as epoolA, \
         tc.tile_pool(name="epoolB", bufs=2 * TC) as epoolB, \
         tc.tile_pool(name="small", bufs=4) as spool, \
         tc.tile_pool(name="pvpool", bufs=QC) as pvpool, \
         tc.tile_pool(name="ps_score", bufs=2, space="PSUM") as ps_score, \
         tc.tile_pool(name="ps_pv", bufs=QC, space="PSUM") as ps_pv:

        # ---- constants / staging tiles ----
        xt_sb = [cpool.tile([P, S], BF16, name=f"xt{kc}") for kc in range(KCH)]
        wqk_sb = cpool.tile([P, KCH, 6 * D], BF16)
        wv_sb = cpool.tile([P, KCH, HPC * D], BF16)
        wp0_sb = cpool.tile([P, NX], BF16)
        wp1_sb = cpool.tile([P, NX], BF16)  # head-2 rows 0:64; 64:128 zero
        bq_sb = cpool.tile([P, HPC], F32)
        bk_sb = cpool.tile([P, HPC], F32)
        ones4 = cpool.tile([97, D], F32)
        scratch = cpool.tile([97, D], BF16)

        q2_sb = cpool.tile([P, HPC, S], BF16)
        # K^T stored zero-padded to 128 contraction rows so the scores
        # matmuls keep Fast Weight Load (needs a full 128-partition lhsT):
        # kA holds even token-chunks in rows 0:64 (rows 64:128 zero), kB
        # holds odd chunks in rows 64:128 (rows 0:64 zero). q2 is
        # duplicated in both halves, so the zero rows contribute nothing.
        kA_sb = cpool.tile([P, HPC, S // 2], BF16)
        kB_sb = cpool.tile([P, HPC, S // 2], BF16)
        v_sb = cpool.tile([P, TC, HPC + 1, D + 1], BF16)
        aT_ab = cpool.tile([P, S], BF16)   # heads 0,1 stacked
        aT_c = cpool.tile([P, S], BF16)    # head 2 in rows 0:64, rest zero

        nc.vector.memset(ones4[:], 1.0)
        nc.gpsimd.memset(kA_sb[D:P], 0.0)
        nc.gpsimd.memset(kB_sb[0:D], 0.0)
        nc.gpsimd.memset(aT_c[D:P], 0.0)
        nc.gpsimd.memset(wp1_sb[D:P], 0.0)
        nc.gpsimd.memset(v_sb[:, :, HPC, :], 0.0)

        # ---- input DMAs, chunked so compute can start early ----
        xt_r = xt_d.rearrange("(c p) s -> c p s", p=P)
        nc.sync.dma_start(xt_sb[0][:], xt_r[0])
        nc.scalar.dma_start(wqk_sb[:],
                            wqk_d.rearrange("(c p) m -> p c m", p=P))
        for kc in range(1, KCH):
            eng = nc.sync if kc % 2 else nc.scalar
            eng.dma_start(xt_sb[kc][:], xt_r[kc])
        nc.sync.dma_start(bq_sb[:], bq_d)
        nc.sync.dma_start(bk_sb[:], bk_d)
        nc.scalar.dma_start(wv_sb[:], wv_d.rearrange("(c p) m -> p c m", p=P))
        nc.sync.dma_start(wp0_sb[:], wp_d[0:P, :])
        nc.sync.dma_start(wp1_sb[0:D], wp_d[P:HPC * D, :])

        # preload the ACT exp table during the DMA window (one-time ~2.7us)
        nc.scalar.activation(scratch[:], ones4[:],
                             mybir.ActivationFunctionType.Exp, scale=0.125)

        # wqk col order is [qA kA qB kB qC kC]; m-chunk mc covers head mc's
        # q (psum partitions 0:64) and k (64:128). Emits Q^T/K^T directly.
        # q2: Q^T duplicated into both partition halves (rows 0:64 == 64:128)
        # k2: K^T with even token-chunks in rows 0:64, odd in rows 64:128 —
        # the stationary layout for the row-tiled (T0/T8) scores matmuls.
        def qk_proj(mc):
            for qc in range(QC):
                ps = ps_pv.tile([P, 512], F32, tag="pv", name=f"qk{mc}_{qc}")
                for kc in range(KCH):
                    nc.tensor.matmul(
                        ps[:],
                        wqk_sb[:, kc, mc * 128:(mc + 1) * 128],
                        xt_sb[kc][:, qc * 512:(qc + 1) * 512],
                        start=(kc == 0), stop=(kc == KCH - 1))
                qsl = slice(qc * 512, (qc + 1) * 512)
                nc.vector.tensor_scalar_add(
                    q2_sb[0:D, mc, qsl], ps[0:D, :], bq_sb[0:D, mc:mc + 1])
                nc.vector.tensor_copy(q2_sb[D:P, mc, qsl],
                                      q2_sb[0:D, mc, qsl])
                kview = ps[D:P, :].rearrange("p (b c) -> p b c", c=128)
                kAw = kA_sb[0:D, mc, qc * 256:(qc + 1) * 256].rearrange(
                    "p (b c) -> p b c", c=128)
                kBw = kB_sb[D:P, mc, qc * 256:(qc + 1) * 256].rearrange(
                    "p (b c) -> p b c", c=128)
                nc.scalar.activation(kAw, kview[:, 0::2, :],
                                     mybir.ActivationFunctionType.Identity,
                                     bias=bk_sb[0:D, mc:mc + 1])
                nc.scalar.activation(kBw, kview[:, 1::2, :],
                                     mybir.ActivationFunctionType.Identity,
                                     bias=bk_sb[D:P, mc:mc + 1])

        def v_proj_chunk(t):
            if t == 0:
                nc.vector.memset(v_sb[:, :, 0:HPC, D:D + 1], 1.0)
            ps = ps_pv.tile([P, 512], F32, tag="pv", name=f"v_{t}")
            for kc in range(KCH):
                nc.tensor.matmul(
                    ps[:, 0:HPC * D],
                    xt_sb[kc][:, t * 128:(t + 1) * 128],
                    wv_sb[:, kc, :],
                    start=(kc == 0), stop=(kc == KCH - 1))
            nc.vector.tensor_copy(
                v_sb[:, t, 0:HPC, 0:D],
                ps[:, 0:HPC * D].rearrange("p (h d) -> p h d", h=HPC))

        e_tiles = {}

        # Schraudolph bf16 exp bits for the DVE-offloaded tiles:
        # bits = round(score * 0.125 * 128/ln2 + (127*128 - 5.42)); the
        # -5.42 centers the piecewise-linear mantissa error (~2.1% rms,
        # validated on HW: DVE f32->int16 converts round-to-nearest).
        SCH_SCALE = 0.125 * 128.0 / float(np.log(2.0))
        SCH_BIAS = 127.0 * 128.0 - 5.42

        def exp_tile(dst, ps, approx):
            if approx:
                nc.vector.tensor_scalar(
                    dst.bitcast(mybir.dt.int16), ps, SCH_SCALE, SCH_BIAS,
                    op0=mybir.AluOpType.mult, op1=mybir.AluOpType.add)
            else:
                nc.scalar.activation(dst, ps,
                                     mybir.ActivationFunctionType.Exp,
                                     scale=0.125)

        def scores_exp_pair(h, j):
            # two t-chunks (2j: kA even chunk, 2j+1: kB odd chunk); e tiles
            # split by query half (q01 pool is short-lived, q23 lives until
            # the second PV pass in W3). A quarter of the exps go to the
            # DVE via the Schraudolph approximation to unload ACT.
            eA01 = epoolA.tile([P, 1024], BF16, tag="E01", name=f"eA01_{h}_{j}")
            eB01 = epoolA.tile([P, 1024], BF16, tag="E01", name=f"eB01_{h}_{j}")
            eA23 = epoolB.tile([P, 1024], BF16, tag="E23", name=f"eA23_{h}_{j}")
            eB23 = epoolB.tile([P, 1024], BF16, tag="E23", name=f"eB23_{h}_{j}")
            e_tiles[(h, 2 * j)] = (eA01, eA23)
            e_tiles[(h, 2 * j + 1)] = (eB01, eB23)
            for half in range(NHALF):
                psA = ps_score.tile([P, 1024], F32, tag="score", name="psA")
                psB = ps_score.tile([P, 1024], F32, tag="score", name="psB")
                for qq in range(2):
                    qsl = slice((half * 2 + qq) * 512,
                                (half * 2 + qq + 1) * 512)
                    nc.tensor.matmul(
                        psA[:, qq * 512:(qq + 1) * 512],
                        kA_sb[:, h, j * 128:(j + 1) * 128],
                        q2_sb[:, h, qsl], start=True, stop=True)
                    nc.tensor.matmul(
                        psB[:, qq * 512:(qq + 1) * 512],
                        kB_sb[:, h, j * 128:(j + 1) * 128],
                        q2_sb[:, h, qsl], start=True, stop=True)
                offl = (half == j % 2) and h > 0
                exp_tile(eA01[:] if half == 0 else eA23[:], psA[:], offl)
                exp_tile(eB01[:] if half == 0 else eB23[:], psB[:], False)

        def pv_chunk(h, t, pvs, qcs):
            e01, e23 = e_tiles[(h, t)]
            vflat = v_sb[:, t].rearrange("p h d -> p (h d)")
            vw = vflat[:, h * (D + 1):h * (D + 1) + P]
            for qc in qcs:
                e = e01 if qc < 2 else e23
                off = (qc % 2) * 512
                nc.tensor.matmul(
                    pvs[qc][:],
                    vw,
                    e[:, off:off + 512],
                    start=(t == 0), stop=(t == TC - 1))

        def norm_copies(entries):
            # Per entry, the two copies (sumexp row + the 64 PV rows) are
            # the ONLY readers of the PV PSUM tile, so it frees immediately
            # and the next window's PV accumulation can claim the bank.
            rt = spool.tile([97, 512], F32, tag="rt")
            nc.vector.memset(rt[:], 1.0)
            stage = []
            for i, (h, pvt, qc, row) in enumerate(entries):
                nc.vector.tensor_copy(rt[row:row + 1, :], pvt[D:D + 1, :])
                pvsb = pvpool.tile([D, 512], F32, tag="pvsb",
                                   name=f"pvsb_{h}_{qc}")
                if i % 2 == 0:
                    nc.vector.tensor_copy(pvsb[:], pvt[0:D, :])
                else:
                    nc.scalar.copy(pvsb[:], pvt[0:D, :])
                stage.append(pvsb)
            return rt, stage

        def norm_finish(entries, rt, stage):
            rr = spool.tile([97, 512], F32, tag="rr")
            nc.vector.reciprocal_approx_fast(rr[:], rt[:])
            for (h, pvt, qc, row), pvsb in zip(entries, stage):
                rb = ps_score.tile([P, 1024], F32, tag="score", name="rb")
                nc.tensor.matmul(rb[0:D, 0:512],
                                 ones4[row:row + 1, :],
                                 rr[row:row + 1, :],
                                 start=True, stop=True,
                                 tile_position=(row, 0))
                dst = (aT_ab[h * D:(h + 1) * D, qc * 512:(qc + 1) * 512]
                       if h < 2 else aT_c[0:D, qc * 512:(qc + 1) * 512])
                nc.vector.tensor_tensor(dst, pvsb[:], rb[0:D, 0:512],
                                        mybir.AluOpType.mult)

        def norm_batch(entries):
            rt, stage = norm_copies(entries)
            norm_finish(entries, rt, stage)

        # output projection chunk: out[sc*128:(sc+1)*128, :] partial -> DRAM
        # (bf16). PSUM->SBUF staging alternates Vector/Scalar engines.
        def proj_chunk(sc):
            s_sl = slice(sc * 128, (sc + 1) * 128)
            ps = ps_score.tile([P, 1024], F32, tag="score",
                               name=f"proj_{sc}")
            for n0, nw in ((0, 512), (512, 256)):
                nc.tensor.matmul(ps[:, n0:n0 + nw], aT_ab[:, s_sl],
                                 wp0_sb[:, n0:n0 + nw],
                                 start=True, stop=False)
                nc.tensor.matmul(ps[:, n0:n0 + nw], aT_c[:, s_sl],
                                 wp1_sb[:, n0:n0 + nw],
                                 start=False, stop=True)
            ostage = spool.tile([P, NX], BF16, tag="ostage",
                                name=f"ostage_{sc}")
            # split the evacuation: DVE takes the first PSUM bank, ACT the
            # second (ACT copies slow down sharply when crossing banks)
            nc.vector.tensor_copy(ostage[:, 0:512], ps[:, 0:512])
            nc.scalar.copy(ostage[:, 512:NX], ps[:, 512:NX])
            nc.sync.dma_start(out_d[s_sl, :], ostage[:])

        # ---- emission order = pipeline order ----
        qk_proj(0)

        # W0: scores+exp for head 0 is ACT-bound (~35us); fill the PE with
        # v_proj and the other two qk projections.
        fillers = ([("v", t) for t in range(TC)]
                   + [("qk", 1, qc) for qc in range(QC)]
                   + [("qk", 2, qc) for qc in range(QC)])
        fq = 0

        def emit_fillers(n):
            nonlocal fq
            for _ in range(n):
                if fq >= len(fillers):
                    return
                f = fillers[fq]
                fq += 1
                if f[0] == "v":
                    v_proj_chunk(f[1])
                else:
                    _, mc, qc = f
                    qk_proj_qc(mc, qc)

        # qk_proj split per-qc for finer interleave
        def qk_proj_qc(mc, qc):
            ps = ps_pv.tile([P, 512], F32, tag="pv", name=f"qk{mc}_{qc}")
            for kc in range(KCH):
                nc.tensor.matmul(
                    ps[:],
                    wqk_sb[:, kc, mc * 128:(mc + 1) * 128],
                    xt_sb[kc][:, qc * 512:(qc + 1) * 512],
                    start=(kc == 0), stop=(kc == KCH - 1))
            nc.vector.tensor_scalar_add(
                q2_sb[0:D, mc, qc * 512:(qc + 1) * 512],
                ps[0:D, :], bq_sb[0:D, mc:mc + 1])
            kview = ps[D:P, :].rearrange("p (b c) -> p b c", c=128)
            kAw = kA_sb[0:D, mc, qc * 256:(qc + 1) * 256].rearrange(
                "p (b c) -> p b c", c=128)
            kBw = kB_sb[D:P, mc, qc * 256:(qc + 1) * 256].rearrange(
                "p (b c) -> p b c", c=128)
            nc.vector.tensor_scalar_add(kAw, kview[:, 0::2, :],
                                        bk_sb[0:D, mc:mc + 1])
            nc.vector.tensor_scalar_add(kBw, kview[:, 1::2, :],
                                        bk_sb[D:P, mc:mc + 1])
            if qc == QC - 1:
                nc.vector.tensor_copy(q2_sb[D:P, mc, :], q2_sb[0:D, mc, :])

        for j in range(TC // 2):
            scores_exp_pair(0, j)
            # wait-floor hint: keep the fillers from being list-scheduled
            # ahead of the first scores pairs (which would delay the first
            # exps and with them the whole ACT stream)
            if j < 3:
                with tc.tile_wait_until(0.020 + 0.0042 * j):
                    emit_fillers(3)
            else:
                emit_fillers(3)
        emit_fillers(len(fillers))  # any stragglers

        # W1: PV(h0, all qc) interleaved with scores+exp for head 1.
        pvs0 = [ps_pv.tile([P, 512], F32, tag="pv", name=f"pv_0_{qc}")
                for qc in range(QC)]
        for t in range(TC):
            if t % 2 == 0 and t < 4:
                scores_exp_pair(1, t // 2)
            pv_chunk(0, t, pvs0, range(QC))
            if t % 2 == 0 and t >= 4:
                scores_exp_pair(1, t // 2)
        ent0 = [(0, pvs0[qc], qc, 32 * qc) for qc in range(QC)]
        rt0, stage0 = norm_copies(ent0)
        with tc.tile_wait_until(0.115):
            norm_finish(ent0, rt0, stage0)

        # W2: PV(h1, qc01) + scores for head 2, with PV(h2, qc01) lagging
        # four t-chunks behind (its e tiles come from the h2 exps landing in
        # this same window). Uses 2+2 PV PSUM banks + 4 scores banks = 8.
        h1a = {qc: ps_pv.tile([P, 512], F32, tag="pv", name=f"pv_1_{qc}")
               for qc in (0, 1)}
        h2a = {qc: ps_pv.tile([P, 512], F32, tag="pv", name=f"pv_2_{qc}")
               for qc in (0, 1)}
        for t in range(TC):
            if t % 2 == 0 and t < 4:
                scores_exp_pair(2, t // 2)
            pv_chunk(1, t, h1a, (0, 1))
            if t % 2 == 0 and t >= 4:
                scores_exp_pair(2, t // 2)
            if t >= 4:
                pv_chunk(2, t - 4, h2a, (0, 1))
        for tt in range(TC - 4, TC):
            pv_chunk(2, tt, h2a, (0, 1))
        norm_batch([(1, h1a[0], 0, 0), (1, h1a[1], 1, 32),
                    (2, h2a[0], 0, 64), (2, h2a[1], 1, 96)])

        # W3: qc2 pass for heads 1+2 with proj chunks 0..7 (qc01 s-range)
        # interleaved, then the qc3 pass with proj 8..11 (qc2 s-range), so
        # only chunks 12..15 trail the last norm.
        h1b = {2: ps_pv.tile([P, 512], F32, tag="pv", name="pv_1_2")}
        h2b = {2: ps_pv.tile([P, 512], F32, tag="pv", name="pv_2_2")}
        for t in range(TC):
            pv_chunk(1, t, h1b, (2,))
            pv_chunk(2, t, h2b, (2,))
            if t % 2 == 1:
                proj_chunk(t // 2)
        norm_batch([(1, h1b[2], 2, 0), (2, h2b[2], 2, 64)])
        h1c = {3: ps_pv.tile([P, 512], F32, tag="pv", name="pv_1_3")}
        h2c = {3: ps_pv.tile([P, 512], F32, tag="pv", name="pv_2_3")}
        for t in range(TC):
            pv_chunk(1, t, h1c, (3,))
            pv_chunk(2, t, h2c, (3,))
            if t % 2 == 1 and t >= 7 and 8 + (t - 7) // 2 < 12:
                proj_chunk(8 + (t - 7) // 2)
        norm_batch([(1, h1c[3], 3, 32), (2, h2c[3], 3, 96)])
        for sc in range(12, S // 128):
            proj_chunk(sc)


# ---------------------------------------------------------------------------
# host side
# ---------------------------------------------------------------------------

def make_in_maps(hidden_states, w_attn, b_attn, w_proj, S=2048):
    """Build the 8 per-core input dicts (numpy bf16)."""
    bf = ml_dtypes.bfloat16
    hidden = np.asarray(hidden_states)
    w_attn = np.asarray(w_attn, dtype=np.float32)
    b_attn = np.asarray(b_attn, dtype=np.float32)
    w_proj = np.asarray(w_proj, dtype=np.float32)

    xts = [np.ascontiguousarray(hidden[b].T).astype(bf)
           for b in range(hidden.shape[0])]

    in_maps = []
    for c in range(N_CORES):
        b = c // (N_CORES // hidden.shape[0])
        h0 = HPC * (c % (N_CORES // hidden.shape[0]))
        wqk = np.zeros((KDIM, 6 * D), dtype=bf)
        wv = np.zeros((KDIM, HPC * D), dtype=bf)
        bq = np.zeros((128, HPC), dtype=np.float32)
        bk = np.zeros((128, HPC), dtype=np.float32)
        for i in range(HPC):
            h = h0 + i
            wqk[:, (2 * i) * D:(2 * i + 1) * D] = \
                w_attn[:, h * D:(h + 1) * D].astype(bf)
            wqk[:, (2 * i + 1) * D:(2 * i + 2) * D] = \
                w_attn[:, NX + h * D:NX + (h + 1) * D].astype(bf)
            wv[:, i * D:(i + 1) * D] = \
                w_attn[:, 2 * NX + h * D:2 * NX + (h + 1) * D].astype(bf)
            bq[0:D, i] = b_attn[h * D:(h + 1) * D]
            bk[0:D, i] = b_attn[NX + h * D:NX + (h + 1) * D]
            bk[D:128, i] = b_attn[NX + h * D:NX + (h + 1) * D]
        wp = w_proj[h0 * D:(h0 + HPC) * D, :].astype(bf)
        in_maps.append({"xt": xts[b], "wqk": wqk, "wv": wv, "wp": wp,
                        "bq": bq, "bk": bk})
    return in_maps


def gather_out(results, hidden_shape, b_attn, w_proj, b_proj):
    """Sum per-core bf16 partials -> [B, S, NX] f32, adding the folded
    bias (b_proj + b_v @ w_proj, exact since v-bias passes softmax as a
    constant)."""
    B, S, _ = hidden_shape
    cpb = N_CORES // B
    out = np.zeros((B, S, NX), dtype=np.float32)
    for c in range(N_CORES):
        out[c // cpb] += np.asarray(results[c]["out"], dtype=np.float32)
    b_attn = np.asarray(b_attn, dtype=np.float32)
    b_eff = (np.asarray(b_proj, dtype=np.float32)
             + b_attn[2 * NX:] @ np.asarray(w_proj, dtype=np.float32))
    out += b_eff
    return out


_CACHE = {}


def kernel(hidden_states, w_attn, b_attn, w_proj, b_proj):
    from concourse.bass_utils import run_bass_kernel_spmd

    hidden = np.asarray(hidden_states, dtype=np.float32)
    B, S, _ = hidden.shape
    in_maps = make_in_maps(hidden, w_attn, b_attn, w_proj, S=S)

    if S not in _CACHE:
        _CACHE[S] = build_nc(S=S)
    nc = _CACHE[S]

    res = run_bass_kernel_spmd(nc, in_maps, core_ids=list(range(N_CORES)))
    return gather_out(res.results, hidden.shape, b_attn, w_proj, b_proj)


# revision 27
# speedup vs baseline: 1.0201x; 1.0201x over previous
"""Multi-head attention (B=2, S=2048, nx=768, H=12) on 8 TRN2 NeuronCores.

Sharding: 24 (batch, head) pairs -> 3 heads per core. Core c handles batch
c//4, heads {3*(c%4), +1, +2}. Each core computes QKV projection for its
head slice, attention, and a partial output projection (its 192 rows of
w_proj); the host sums the 4 partials per batch and adds the bias.

Schedule (~189us vs 241us baseline): the scalar-engine exp stream (96 x
[128,1024] activations, ~20% offloaded to the DVE) is the metronome; all
PE work hides inside it. Key points:
  - contraction is 6x128 (= nx) with no bias row: v-bias folds exactly into
    the host-side output bias (b_eff = b_proj + b_v @ w_proj); q/k biases
    are added during the PSUM->SBUF copies as per-partition scalars.
  - all stationary operands are padded to full 128 partitions / 128
    columns so every matmul keeps Fast Weight Load (K=64 operands pay
    ~110ns of exposed LDWEIGHTS otherwise): kA=[K_even;0], kB=[0;K_odd]
    with q duplicated into both halves, and the PV lhsT view spans
    v|ones|next-head-spill (PSUM rows 65:128 take junk, never read).
  - a fifth of the exps run on the DVE as a one-instruction Schraudolph
    approximation (f32->int16 round-to-nearest affine = bf16 exp bits,
    ~2% rms), which the accuracy budget absorbs (rel err 8.8e-3 vs 2e-2).
  - chunked input DMAs across both HWDGE queues (sync + scalar); v_proj /
    qk_proj(1,2) fill the head-0 scores window; head-2 PV (queries 0:1024)
    lags four chunks behind its exps inside the head-1 window; the
    remaining PV runs qc-merged with the output projection interleaved.
  - norms batch 1/sumexp rows at 32-aligned partitions into one fast
    reciprocal; the PV PSUM is freed by two copies (DVE/ACT alternating)
    before the broadcast multiply, so the next window's accumulation can
    claim the banks immediately.
  - output is staged bf16 (halves the write traffic); the host sums the
    four per-batch partials in f32.
"""

import numpy as np
import ml_dtypes

import concourse.bass as bass
import concourse.tile as tile
import concourse.mybir as mybir
from concourse import bacc

BF16 = mybir.dt.bfloat16
F32 = mybir.dt.float32

NX = 768
D = 64
HPC = 3          # heads per core
N_CORES = 8
KCH = 6          # contraction chunks of 128 (= nx, no bias row)
KDIM = KCH * 128  # 768


def build_nc(S=2048):
    """Build the single-core SPMD program. S = sequence length."""
    TC = S // 128    # t (key) chunks
    QC = S // 512    # q chunks of 512
    nc = bacc.Bacc("TRN2", target_bir_lowering=False, debug=False)

    xt_d = nc.dram_tensor("xt", [KDIM, S], BF16, kind="ExternalInput")
    wqk_d = nc.dram_tensor("wqk", [KDIM, 6 * D], BF16, kind="ExternalInput")
    wv_d = nc.dram_tensor("wv", [KDIM, HPC * D], BF16, kind="ExternalInput")
    wp_d = nc.dram_tensor("wp", [HPC * D, NX], BF16, kind="ExternalInput")
    bq_d = nc.dram_tensor("bq", [128, HPC], F32, kind="ExternalInput")
    bk_d = nc.dram_tensor("bk", [128, HPC], F32, kind="ExternalInput")
    out_d = nc.dram_tensor("out", [S, NX], BF16, kind="ExternalOutput")

    with tile.TileContext(nc) as tc:
        _build_body(tc, out_d.ap(), xt_d.ap(), wqk_d.ap(), wv_d.ap(),
                    wp_d.ap(), bq_d.ap(), bk_d.ap(), S, TC, QC)
    nc.compile()
    return nc


def _build_body(tc, out_d, xt_d, wqk_d, wv_d, wp_d, bq_d, bk_d, S, TC, QC):
    nc = tc.nc
    P = 128
    NHALF = S // 1024  # exp calls per t-chunk, each [128, 1024]

    with tc.tile_pool(name="const", bufs=1) as cpool, \
         tc.tile_pool(name="epoolA", bufs=TC + 2) as epoolA, \
         tc.tile_pool(name="epoolB", bufs=2 * TC) as epoolB, \
         tc.tile_pool(name="small", bufs=4) as spool, \
         tc.tile_pool(name="pvpool", bufs=QC) as pvpool, \
         tc.tile_pool(name="ps_score", bufs=2, space="PSUM") as ps_score, \
         tc.tile_pool(name="ps_pv", bufs=QC, space="PSUM") as ps_pv:

        # ---- constants / staging tiles ----
        xt_sb = [cpool.tile([P, S], BF16, name=f"xt{kc}") for kc in range(KCH)]
        wqk_sb = cpool.tile([P, KCH, 6 * D], BF16)
        wv_sb = cpool.tile([P, KCH, HPC * D], BF16)
        wp0_sb = cpool.tile([P, NX], BF16)
        wp1_sb = cpool.tile([P, NX], BF16)  # head-2 rows 0:64; 64:128 zero
        bq_sb = cpool.tile([P, HPC], F32)
        bk_sb = cpool.tile([P, HPC], F32)
        ones4 = cpool.tile([97, D], F32)
        scratch = cpool.tile([97, D], BF16)

        q2_sb = cpool.tile([P, HPC, S], BF16)
        # K^T stored zero-padded to 128 contraction rows so the scores
        # matmuls keep Fast Weight Load (needs a full 128-partition lhsT):
        # kA holds even token-chunks in rows 0:64 (rows 64:128 zero), kB
        # holds odd chunks in rows 64:128 (rows 0:64 zero). q2 is
        # duplicated in both halves, so the zero rows contribute nothing.
        kA_sb = cpool.tile([P, HPC, S // 2], BF16)
        kB_sb = cpool.tile([P, HPC, S // 2], BF16)
        v_sb = cpool.tile([P, TC, HPC + 1, D + 1], BF16)
        aT_ab = cpool.tile([P, S], BF16)   # heads 0,1 stacked
        aT_c = cpool.tile([P, S], BF16)    # head 2 in rows 0:64, rest zero

        nc.vector.memset(ones4[:], 1.0)
        nc.gpsimd.memset(kA_sb[D:P], 0.0)
        nc.gpsimd.memset(kB_sb[0:D], 0.0)
        nc.gpsimd.memset(aT_c[D:P], 0.0)
        nc.gpsimd.memset(wp1_sb[D:P], 0.0)
        nc.gpsimd.memset(v_sb[:, :, HPC, :], 0.0)

        # ---- input DMAs, chunked so compute can start early ----
        xt_r = xt_d.rearrange("(c p) s -> c p s", p=P)
        nc.sync.dma_start(xt_sb[0][:], xt_r[0])
        nc.scalar.dma_start(wqk_sb[:],
                            wqk_d.rearrange("(c p) m -> p c m", p=P))
        for kc in range(1, KCH):
            eng = nc.sync if kc % 2 else nc.scalar
            eng.dma_start(xt_sb[kc][:], xt_r[kc])
        nc.sync.dma_start(bq_sb[:], bq_d)
        nc.sync.dma_start(bk_sb[:], bk_d)
        nc.scalar.dma_start(wv_sb[:], wv_d.rearrange("(c p) m -> p c m", p=P))
        nc.sync.dma_start(wp0_sb[:], wp_d[0:P, :])
        nc.sync.dma_start(wp1_sb[0:D], wp_d[P:HPC * D, :])

        # preload the ACT exp table during the DMA window (one-time ~2.7us)
        nc.scalar.activation(scratch[:], ones4[:],
                             mybir.ActivationFunctionType.Exp, scale=0.125)

        # wqk col order is [qA kA qB kB qC kC]; m-chunk mc covers head mc's
        # q (psum partitions 0:64) and k (64:128). Emits Q^T/K^T directly.
        # q2: Q^T duplicated into both partition halves (rows 0:64 == 64:128)
        # k2: K^T with even token-chunks in rows 0:64, odd in rows 64:128 —
        # the stationary layout for the row-tiled (T0/T8) scores matmuls.
        def qk_proj(mc):
            for qc in range(QC):
                ps = ps_pv.tile([P, 512], F32, tag="pv", name=f"qk{mc}_{qc}")
                for kc in range(KCH):
                    nc.tensor.matmul(
                        ps[:],
                        wqk_sb[:, kc, mc * 128:(mc + 1) * 128],
                        xt_sb[kc][:, qc * 512:(qc + 1) * 512],
                        start=(kc == 0), stop=(kc == KCH - 1))
                qsl = slice(qc * 512, (qc + 1) * 512)
                nc.vector.tensor_scalar_add(
                    q2_sb[0:D, mc, qsl], ps[0:D, :], bq_sb[0:D, mc:mc + 1])
                nc.vector.tensor_copy(q2_sb[D:P, mc, qsl],
                                      q2_sb[0:D, mc, qsl])
                kview = ps[D:P, :].rearrange("p (b c) -> p b c", c=128)
                kAw = kA_sb[0:D, mc, qc * 256:(qc + 1) * 256].rearrange(
                    "p (b c) -> p b c", c=128)
                kBw = kB_sb[D:P, mc, qc * 256:(qc + 1) * 256].rearrange(
                    "p (b c) -> p b c", c=128)
                nc.scalar.activation(kAw, kview[:, 0::2, :],
                                     mybir.ActivationFunctionType.Identity,
                                     bias=bk_sb[0:D, mc:mc + 1])
                nc.scalar.activation(kBw, kview[:, 1::2, :],
                                     mybir.ActivationFunctionType.Identity,
                                     bias=bk_sb[D:P, mc:mc + 1])

        def v_proj_chunk(t):
            if t == 0:
                nc.vector.memset(v_sb[:, :, 0:HPC, D:D + 1], 1.0)
            ps = ps_pv.tile([P, 512], F32, tag="pv", name=f"v_{t}")
            for kc in range(KCH):
                nc.tensor.matmul(
                    ps[:, 0:HPC * D],
                    xt_sb[kc][:, t * 128:(t + 1) * 128],
                    wv_sb[:, kc, :],
                    start=(kc == 0), stop=(kc == KCH - 1))
            nc.vector.tensor_copy(
                v_sb[:, t, 0:HPC, 0:D],
                ps[:, 0:HPC * D].rearrange("p (h d) -> p h d", h=HPC))

        e_tiles = {}

        # Schraudolph bf16 exp bits for the DVE-offloaded tiles:
        # bits = round(score * 0.125 * 128/ln2 + (127*128 - 5.42)); the
        # -5.42 centers the piecewise-linear mantissa error (~2.1% rms,
        # validated on HW: DVE f32->int16 converts round-to-nearest).
        SCH_SCALE = 0.125 * 128.0 / float(np.log(2.0))
        SCH_BIAS = 127.0 * 128.0 - 5.42

        def exp_tile(dst, ps, approx):
            if approx:
                nc.vector.tensor_scalar(
                    dst.bitcast(mybir.dt.int16), ps, SCH_SCALE, SCH_BIAS,
                    op0=mybir.AluOpType.mult, op1=mybir.AluOpType.add)
            else:
                nc.scalar.activation(dst, ps,
                                     mybir.ActivationFunctionType.Exp,
                                     scale=0.125)

        def scores_exp_pair(h, j):
            # two t-chunks (2j: kA even chunk, 2j+1: kB odd chunk); e tiles
            # split by query half (q01 pool is short-lived, q23 lives until
            # the second PV pass in W3). A quarter of the exps go to the
            # DVE via the Schraudolph approximation to unload ACT.
            eA01 = epoolA.tile([P, 1024], BF16, tag="E01", name=f"eA01_{h}_{j}")
            eB01 = epoolA.tile([P, 1024], BF16, tag="E01", name=f"eB01_{h}_{j}")
            eA23 = epoolB.tile([P, 1024], BF16, tag="E23", name=f"eA23_{h}_{j}")
            eB23 = epoolB.tile([P, 1024], BF16, tag="E23", name=f"eB23_{h}_{j}")
            e_tiles[(h, 2 * j)] = (eA01, eA23)
            e_tiles[(h, 2 * j + 1)] = (eB01, eB23)
            for half in range(NHALF):
                psA = ps_score.tile([P, 1024], F32, tag="score", name="psA")
                psB = ps_score.tile([P, 1024], F32, tag="score", name="psB")
                for qq in range(2):
                    qsl = slice((half * 2 + qq) * 512,
                                (half * 2 + qq + 1) * 512)
                    nc.tensor.matmul(
                        psA[:, qq * 512:(qq + 1) * 512],
                        kA_sb[:, h, j * 128:(j + 1) * 128],
                        q2_sb[:, h, qsl], start=True, stop=True)
                    nc.tensor.matmul(
                        psB[:, qq * 512:(qq + 1) * 512],
                        kB_sb[:, h, j * 128:(j + 1) * 128],
                        q2_sb[:, h, qsl], start=True, stop=True)
                offl = (half == j % 2) and h > 0
                exp_tile(eA01[:] if half == 0 else eA23[:], psA[:], offl)
                exp_tile(eB01[:] if half == 0 else eB23[:], psB[:], False)

        def pv_chunk(h, t, pvs, qcs):
            e01, e23 = e_tiles[(h, t)]
            vflat = v_sb[:, t].rearrange("p h d -> p (h d)")
            vw = vflat[:, h * (D + 1):h * (D + 1) + P]
            for qc in qcs:
                e = e01 if qc < 2 else e23
                off = (qc % 2) * 512
                nc.tensor.matmul(
                    pvs[qc][:],
                    vw,
                    e[:, off:off + 512],
                    start=(t == 0), stop=(t == TC - 1))

        def norm_copies(entries):
            # Per entry, the two copies (sumexp row + the 64 PV rows) are
            # the ONLY readers of the PV PSUM tile, so it frees immediately
            # and the next window's PV accumulation can claim the bank.
            rt = spool.tile([97, 512], F32, tag="rt")
            nc.vector.memset(rt[:], 1.0)
            stage = []
            for i, (h, pvt, qc, row) in enumerate(entries):
                nc.vector.tensor_copy(rt[row:row + 1, :], pvt[D:D + 1, :])
                pvsb = pvpool.tile([D, 512], F32, tag="pvsb",
                                   name=f"pvsb_{h}_{qc}")
                if i % 2 == 0:
                    nc.vector.tensor_copy(pvsb[:], pvt[0:D, :])
                else:
                    nc.scalar.copy(pvsb[:], pvt[0:D, :])
                stage.append(pvsb)
            return rt, stage

        def norm_finish(entries, rt, stage, gp=False):
            rr = spool.tile([97, 512], F32, tag="rr")
            nc.vector.reciprocal_approx_fast(rr[:], rt[:])
            for (h, pvt, qc, row), pvsb in zip(entries, stage):
                dst = (aT_ab[h * D:(h + 1) * D, qc * 512:(qc + 1) * 512]
                       if h < 2 else aT_c[0:D, qc * 512:(qc + 1) * 512])
                if gp:
                    # broadcast 1/sumexp across partitions on the idle
                    # gpsimd engine (into the dead rt tile) — keeps the
                    # mid-stream norm off the PE and the scores PSUM pool
                    nc.gpsimd.partition_broadcast(rt[0:D, :],
                                                  rr[row:row + 1, :],
                                                  channels=D)
                    nc.vector.tensor_tensor(dst, pvsb[:], rt[0:D, :],
                                            mybir.AluOpType.mult)
                else:
                    rb = ps_score.tile([P, 1024], F32, tag="score",
                                       name="rb")
                    nc.tensor.matmul(rb[0:D, 0:512],
                                     ones4[row:row + 1, :],
                                     rr[row:row + 1, :],
                                     start=True, stop=True,
                                     tile_position=(row, 0))
                    nc.vector.tensor_tensor(dst, pvsb[:], rb[0:D, 0:512],
                                            mybir.AluOpType.mult)

        def norm_batch(entries):
            rt, stage = norm_copies(entries)
            norm_finish(entries, rt, stage)

        # output projection chunk: out[sc*128:(sc+1)*128, :] partial -> DRAM
        # (bf16). PSUM->SBUF staging alternates Vector/Scalar engines.
        def proj_chunk(sc):
            s_sl = slice(sc * 128, (sc + 1) * 128)
            ps = ps_score.tile([P, 1024], F32, tag="score",
                               name=f"proj_{sc}")
            for n0, nw in ((0, 512), (512, 256)):
                nc.tensor.matmul(ps[:, n0:n0 + nw], aT_ab[:, s_sl],
                                 wp0_sb[:, n0:n0 + nw],
                                 start=True, stop=False)
                nc.tensor.matmul(ps[:, n0:n0 + nw], aT_c[:, s_sl],
                                 wp1_sb[:, n0:n0 + nw],
                                 start=False, stop=True)
            ostage = spool.tile([P, NX], BF16, tag="ostage",
                                name=f"ostage_{sc}")
            # split the evacuation: DVE takes the first PSUM bank, ACT the
            # second (ACT copies slow down sharply when crossing banks)
            nc.vector.tensor_copy(ostage[:, 0:512], ps[:, 0:512])
            nc.scalar.copy(ostage[:, 512:NX], ps[:, 512:NX])
            nc.sync.dma_start(out_d[s_sl, :], ostage[:])

        # ---- emission order = pipeline order ----
        qk_proj(0)

        # W0: scores+exp for head 0 is ACT-bound (~35us); fill the PE with
        # v_proj and the other two qk projections.
        fillers = ([("v", t) for t in range(TC)]
                   + [("qk", 1, qc) for qc in range(QC)]
                   + [("qk", 2, qc) for qc in range(QC)])
        fq = 0

        def emit_fillers(n):
            nonlocal fq
            for _ in range(n):
                if fq >= len(fillers):
                    return
                f = fillers[fq]
                fq += 1
                if f[0] == "v":
                    v_proj_chunk(f[1])
                else:
                    _, mc, qc = f
                    qk_proj_qc(mc, qc)

        # qk_proj split per-qc for finer interleave
        def qk_proj_qc(mc, qc):
            ps = ps_pv.tile([P, 512], F32, tag="pv", name=f"qk{mc}_{qc}")
            for kc in range(KCH):
                nc.tensor.matmul(
                    ps[:],
                    wqk_sb[:, kc, mc * 128:(mc + 1) * 128],
                    xt_sb[kc][:, qc * 512:(qc + 1) * 512],
                    start=(kc == 0), stop=(kc == KCH - 1))
            nc.vector.tensor_scalar_add(
                q2_sb[0:D, mc, qc * 512:(qc + 1) * 512],
                ps[0:D, :], bq_sb[0:D, mc:mc + 1])
            kview = ps[D:P, :].rearrange("p (b c) -> p b c", c=128)
            kAw = kA_sb[0:D, mc, qc * 256:(qc + 1) * 256].rearrange(
                "p (b c) -> p b c", c=128)
            kBw = kB_sb[D:P, mc, qc * 256:(qc + 1) * 256].rearrange(
                "p (b c) -> p b c", c=128)
            nc.vector.tensor_scalar_add(kAw, kview[:, 0::2, :],
                                        bk_sb[0:D, mc:mc + 1])
            nc.vector.tensor_scalar_add(kBw, kview[:, 1::2, :],
                                        bk_sb[D:P, mc:mc + 1])
            if qc == QC - 1:
                nc.vector.tensor_copy(q2_sb[D:P, mc, :], q2_sb[0:D, mc, :])

        for j in range(TC // 2):
            scores_exp_pair(0, j)
            # wait-floor hint: keep the fillers from being list-scheduled
            # ahead of the first scores pairs (which would delay the first
            # exps and with them the whole ACT stream)
            if j < 3:
                with tc.tile_wait_until(0.020 + 0.0042 * j):
                    emit_fillers(3)
            else:
                emit_fillers(3)
        emit_fillers(len(fillers))  # any stragglers

        # W1: PV(h0, all qc) interleaved with scores+exp for head 1.
        pvs0 = [ps_pv.tile([P, 512], F32, tag="pv", name=f"pv_0_{qc}")
                for qc in range(QC)]
        for t in range(TC):
            pv_chunk(0, t, pvs0, range(QC))
            if t % 2 == 0:
                scores_exp_pair(1, t // 2)
        norm_batch([(0, pvs0[qc], qc, 32 * qc) for qc in range(QC)])

        # W2: PV(h1, qc01) + scores for head 2, with PV(h2, qc01) lagging
        # four t-chunks behind (its e tiles come from the h2 exps landing in
        # this same window). Uses 2+2 PV PSUM banks + 4 scores banks = 8.
        h1a = {qc: ps_pv.tile([P, 512], F32, tag="pv", name=f"pv_1_{qc}")
               for qc in (0, 1)}
        h2a = {qc: ps_pv.tile([P, 512], F32, tag="pv", name=f"pv_2_{qc}")
               for qc in (0, 1)}
        for t in range(TC):
            pv_chunk(1, t, h1a, (0, 1))
            if t % 2 == 0:
                scores_exp_pair(2, t // 2)
            if t >= 4:
                pv_chunk(2, t - 4, h2a, (0, 1))
        for tt in range(TC - 4, TC):
            pv_chunk(2, tt, h2a, (0, 1))
        norm_batch([(1, h1a[0], 0, 0), (1, h1a[1], 1, 32),
                    (2, h2a[0], 0, 64), (2, h2a[1], 1, 96)])

        # W3: qc2 pass for heads 1+2 with proj chunks 0..7 (qc01 s-range)
        # interleaved, then the qc3 pass with proj 8..11 (qc2 s-range), so
        # only chunks 12..15 trail the last norm.
        h1b = {2: ps_pv.tile([P, 512], F32, tag="pv", name="pv_1_2")}
        h2b = {2: ps_pv.tile([P, 512], F32, tag="pv", name="pv_2_2")}
        for t in range(TC):
            pv_chunk(1, t, h1b, (2,))
            pv_chunk(2, t, h2b, (2,))
            if t % 2 == 1:
                proj_chunk(t // 2)
        norm_batch([(1, h1b[2], 2, 0), (2, h2b[2], 2, 64)])
        h1c = {3: ps_pv.tile([P, 512], F32, tag="pv", name="pv_1_3")}
        h2c = {3: ps_pv.tile([P, 512], F32, tag="pv", name="pv_2_3")}
        for t in range(TC):
            pv_chunk(1, t, h1c, (3,))
            pv_chunk(2, t, h2c, (3,))
            if t % 2 == 1 and t >= 7 and 8 + (t - 7) // 2 < 12:
                proj_chunk(8 + (t - 7) // 2)
        norm_batch([(1, h1c[3], 3, 32), (2, h2c[3], 3, 96)])
        for sc in range(12, S // 128):
            proj_chunk(sc)


# ---------------------------------------------------------------------------
# host side
# ---------------------------------------------------------------------------

def make_in_maps(hidden_states, w_attn, b_attn, w_proj, S=2048):
    """Build the 8 per-core input dicts (numpy bf16)."""
    bf = ml_dtypes.bfloat16
    hidden = np.asarray(hidden_states)
    w_attn = np.asarray(w_attn, dtype=np.float32)
    b_attn = np.asarray(b_attn, dtype=np.float32)
    w_proj = np.asarray(w_proj, dtype=np.float32)

    xts = [np.ascontiguousarray(hidden[b].T).astype(bf)
           for b in range(hidden.shape[0])]

    in_maps = []
    for c in range(N_CORES):
        b = c // (N_CORES // hidden.shape[0])
        h0 = HPC * (c % (N_CORES // hidden.shape[0]))
        wqk = np.zeros((KDIM, 6 * D), dtype=bf)
        wv = np.zeros((KDIM, HPC * D), dtype=bf)
        bq = np.zeros((128, HPC), dtype=np.float32)
        bk = np.zeros((128, HPC), dtype=np.float32)
        for i in range(HPC):
            h = h0 + i
            wqk[:, (2 * i) * D:(2 * i + 1) * D] = \
                w_attn[:, h * D:(h + 1) * D].astype(bf)
            wqk[:, (2 * i + 1) * D:(2 * i + 2) * D] = \
                w_attn[:, NX + h * D:NX + (h + 1) * D].astype(bf)
            wv[:, i * D:(i + 1) * D] = \
                w_attn[:, 2 * NX + h * D:2 * NX + (h + 1) * D].astype(bf)
            bq[0:D, i] = b_attn[h * D:(h + 1) * D]
            bk[0:D, i] = b_attn[NX + h * D:NX + (h + 1) * D]
            bk[D:128, i] = b_attn[NX + h * D:NX + (h + 1) * D]
        wp = w_proj[h0 * D:(h0 + HPC) * D, :].astype(bf)
        in_maps.append({"xt": xts[b], "wqk": wqk, "wv": wv, "wp": wp,
                        "bq": bq, "bk": bk})
    return in_maps


def gather_out(results, hidden_shape, b_attn, w_proj, b_proj):
    """Sum per-core bf16 partials -> [B, S, NX] f32, adding the folded
    bias (b_proj + b_v @ w_proj, exact since v-bias passes softmax as a
    constant)."""
    B, S, _ = hidden_shape
    cpb = N_CORES // B
    out = np.zeros((B, S, NX), dtype=np.float32)
    for c in range(N_CORES):
        out[c // cpb] += np.asarray(results[c]["out"], dtype=np.float32)
    b_attn = np.asarray(b_attn, dtype=np.float32)
    b_eff = (np.asarray(b_proj, dtype=np.float32)
             + b_attn[2 * NX:] @ np.asarray(w_proj, dtype=np.float32))
    out += b_eff
    return out


_CACHE = {}


def kernel(hidden_states, w_attn, b_attn, w_proj, b_proj):
    from concourse.bass_utils import run_bass_kernel_spmd

    hidden = np.asarray(hidden_states, dtype=np.float32)
    B, S, _ = hidden.shape
    in_maps = make_in_maps(hidden, w_attn, b_attn, w_proj, S=S)

    if S not in _CACHE:
        _CACHE[S] = build_nc(S=S)
    nc = _CACHE[S]

    res = run_bass_kernel_spmd(nc, in_maps, core_ids=list(range(N_CORES)))
    return gather_out(res.results, hidden.shape, b_attn, w_proj, b_proj)
